# revision 1
# baseline (speedup 1.0000x reference)
"""RWKV-7 TimeMix kernel for 8 Trainium2 NeuronCores.

Sharding: data-parallel over B (8 batches -> 8 cores). Each core runs the
full per-batch module: time-shift lerps, r/k/v projections, LoRA branches
(decay/iclr/gate), the WKV state recurrence (chunked UT-transform with a
truncated-Neumann intra-chunk solve), bonus, GroupNorm, output gate, W_o.

Chunked WKV math per head (chunk L=128, state S[i,j], c = in-chunk cumprod
of the decay d):
  Wt[t] = a_t*kn_t*c_{t-1}      Kn[s] = kn_s/c_s       Vs[s] = v_s/c_s
  G  = triu(Kn Wt^T, 1)   Av = triu(Vs Wt^T, 1)   P = triu(kr r^T, 0)
  B  = Wt S0 + Av^T kr
  (I+G^T) U = B  solved by K Neumann steps  X <- B - G^T X;  Um := -U
  Qm = triu(Um r^T, 0)
  O[t] = c_t * (r S0^T + P^T Vs + Qm^T Kn)
  S   <- diag(c_L) (S + Vs^T kr + Kn^T Um)
Value-path matmuls run fp32 (exact); the G path (G generation + Neumann
applies) runs bf16; big projections run float32r (tf32-like, full speed).
W_k/W_v/W_o are streamed from HBM per super-chunk; W_r stays resident.
"""
import numpy as np

B, T, C, H, N = 8, 2048, 1024, 16, 64
LORA = 64
P = 128
NCT = C // P          # 8 channel tiles
CH = 128              # WKV chunk
SUP = 256             # projection super-chunk
NSUP = T // SUP       # 8
NCH = SUP // CH       # 2
NEUMANN_K = 6
DECAY_SCALE = float(np.exp(-0.5))
GN_EPS = 1e-5 * H
NORM_EPS = 1e-12

VEC_NAMES = ["mu_r", "mu_k", "mu_v", "mu_g", "mu_a", "mu_d",
             "decay_bias", "iclr_bias", "removal_key_multiplier",
             "iclr_mix_amt", "bonus_multiplier", "ln_w", "ln_b"]
MAT_NAMES = ["W_r", "W_k", "W_v", "W_o", "decay_A", "iclr_A", "gate_A",
             "decay_B", "iclr_B", "gate_B"]

_CACHE = {}


def _build():
    import concourse.bass as bass  # noqa: F401
    from concourse import bacc, mybir
    import concourse.tile as tile

    f32 = mybir.dt.float32
    nc = bacc.Bacc("TRN2", target_bir_lowering=False, debug=False, num_devices=B)
    x_h = nc.dram_tensor("x", [T, C], f32, kind="ExternalInput")
    w_h = {n: nc.dram_tensor(n, [C, C], f32, kind="ExternalInput")
           for n in ("W_r", "W_k", "W_v", "W_o")}
    la_h = {n: nc.dram_tensor(n, [C, LORA], f32, kind="ExternalInput")
            for n in ("decay_A", "iclr_A", "gate_A")}
    lb_h = {n: nc.dram_tensor(n, [LORA, C], f32, kind="ExternalInput")
            for n in ("decay_B", "iclr_B", "gate_B")}
    vec_h = {n: nc.dram_tensor(n, [C], f32, kind="ExternalInput") for n in VEC_NAMES}
    y_h = nc.dram_tensor("y", [T, C], f32, kind="ExternalOutput")
    vp_h = nc.dram_tensor("vp", [T, C], f32, kind="ExternalOutput")
    with tile.TileContext(nc) as tc:
        _emit(nc, tc, x_h, w_h, la_h, lb_h, vec_h, y_h, vp_h)
    nc.finalize()
    return nc


def _emit(nc, tc, x_h, w_h, la_h, lb_h, vec_h, y_h, vp_h):
    import concourse.bass as bass
    from concourse import mybir
    from concourse.masks import make_identity
    from contextlib import ExitStack

    f32 = mybir.dt.float32
    f32r = mybir.dt.float32r
    bf16 = mybir.dt.bfloat16
    AF = mybir.ActivationFunctionType
    OP = mybir.AluOpType

    ctx = ExitStack()
    const = ctx.enter_context(tc.tile_pool(name="const", bufs=1))
    supp = ctx.enter_context(tc.tile_pool(name="supp", bufs=1))
    chkp = ctx.enter_context(tc.tile_pool(name="chkp", bufs=1))
    jit1 = ctx.enter_context(tc.tile_pool(name="jit1", bufs=1))
    jit2 = ctx.enter_context(tc.tile_pool(name="jit2", bufs=2))
    jit3 = ctx.enter_context(tc.tile_pool(name="jit3", bufs=3))
    pairp = ctx.enter_context(tc.tile_pool(name="pairp", bufs=2))
    ps_proj = ctx.enter_context(tc.tile_pool(name="ps_proj", bufs=2, space="PSUM"))
    ps_lora = ctx.enter_context(tc.tile_pool(name="ps_lora", bufs=1, space="PSUM"))
    ps_wkv = ctx.enter_context(tc.tile_pool(name="ps_wkv", bufs=3, space="PSUM"))
    ps_small = ctx.enter_context(tc.tile_pool(name="ps_small", bufs=2, space="PSUM"))

    # ---------------- static constants ----------------
    wr_sb = const.tile([P, NCT, C], f32r, tag="wr_sb")
    for i in range(NCT):
        nc.gpsimd.dma_start(out=wr_sb[:, i, :], in_=w_h["W_r"][P * i:P * (i + 1), :])
    vsb = {}
    for n in VEC_NAMES:
        t = const.tile([P, NCT], f32, tag=f"v_{n}", name=f"v_{n}")
        src = vec_h[n][:]
        nc.sync.dma_start(out=t, in_=bass.AP(
            tensor=src.tensor, offset=src.offset, ap=[[1, P], [P, NCT]]))
        vsb[n] = t
    ommix = const.tile([P, NCT], f32, tag="v_ommix")
    nc.vector.tensor_scalar(out=ommix, in0=vsb["iclr_mix_amt"], scalar1=-1.0,
                            scalar2=1.0, op0=OP.mult, op1=OP.add)
    ident = const.tile([P, P], f32, tag="ident")
    make_identity(nc, ident)
    mask_su = const.tile([P, P], f32, tag="mask_su")   # keep s < t  ([s,t])
    nc.gpsimd.memset(mask_su, 1.0)
    nc.gpsimd.affine_select(out=mask_su, in_=mask_su, compare_op=OP.is_gt,
                            fill=0.0, base=0, channel_multiplier=-1,
                            pattern=[[1, P]])
    mask_ui = const.tile([P, P], f32, tag="mask_ui")   # keep s <= t
    nc.gpsimd.memset(mask_ui, 1.0)
    nc.gpsimd.affine_select(out=mask_ui, in_=mask_ui, compare_op=OP.is_ge,
                            fill=0.0, base=0, channel_multiplier=-1,
                            pattern=[[1, P]])
    inds = []
    for i in range(NCT):
        indf = const.tile([P, 16], f32, tag=f"indf{i}", name=f"indf{i}")
        nc.vector.memset(indf, 0.0)
        nc.vector.memset(indf[0:64, 2 * i:2 * i + 1], 1.0)
        nc.vector.memset(indf[64:128, 2 * i + 1:2 * i + 2], 1.0)
        indr = const.tile([P, 16], f32r, tag=f"indr{i}", name=f"indr{i}")
        nc.scalar.copy(out=indr, in_=indf)
        inds.append(indr)
    gn_eps = const.tile([P, 1], f32, tag="gn_eps")
    nc.vector.memset(gn_eps, GN_EPS)
    zeros = const.tile([P, CH], f32, tag="zeros")
    nc.vector.memset(zeros, 0.0)
    S_st = const.tile([64, NCT, 2, 64], f32, tag="S_st")
    St_st = const.tile([64, NCT, 2, 64], f32, tag="St_st")
    nc.vector.memset(S_st, 0.0)
    nc.vector.memset(St_st, 0.0)

    # ------------- per-super persistents -------------
    xext = supp.tile([P, NCT, SUP + 1], f32, tag="xext")
    rT = supp.tile([P, NCT, SUP], f32, tag="rT")
    knT = supp.tile([P, NCT, SUP], f32, tag="knT")
    krT = supp.tile([P, NCT, SUP], f32, tag="krT")
    vT = supp.tile([P, NCT, SUP], f32, tag="vT")
    aT = supp.tile([P, NCT, SUP], f32, tag="aT")
    dT = supp.tile([P, NCT, SUP], f32, tag="dT")
    gateT = supp.tile([P, NCT, SUP], f32, tag="gateT")
    yTs = supp.tile([P, NCT, SUP], f32r, tag="yTs")
    la_out = {n: supp.tile([LORA, SUP], f32r, tag=f"lo_{n}", name=f"lo_{n}")
              for n in ("decay_A", "iclr_A", "gate_A")}
    nb = supp.tile([16, 2 * SUP], f32, tag="nb")
    # ------------- per-chunk persistents -------------
    cext = chkp.tile([P, NCT, CH + 1], f32, tag="cext")
    knt_c = chkp.tile([P, C], f32, tag="knt_c")
    krt_c = chkp.tile([P, C], f32, tag="krt_c")
    cit_c = chkp.tile([P, C], f32, tag="cit_c")
    vp_t = chkp.tile([P, C], f32, tag="vp_t")
    O_c = chkp.tile([P, C], f32, tag="O_c")
    wtf_c = chkp.tile([P, NCT, CH], f32, tag="wtf_c")
    wtb_c = chkp.tile([P, NCT, CH], bf16, tag="wtb_c")
    kntb_c = chkp.tile([P, NCT, CH], bf16, tag="kntb_c")
    vld_c = chkp.tile([P, NCT, CH], f32, tag="vld_c")
    wtlo = chkp.tile([64, NCT, CH], f32, tag="wtlo")
    rtlo = chkp.tile([64, NCT, CH], f32, tag="rtlo")
    cl_al = chkp.tile([64, 2, NCT], f32, tag="cl_al")
    bs_t = chkp.tile([P, 16], f32, tag="bs_t")
    stats6 = chkp.tile([P, 16, 6], f32, tag="stats6")
    mv2 = chkp.tile([P, 16, 2], f32, tag="mv2")
    rstd = chkp.tile([P, 16], f32, tag="rstd")

    tc.strict_bb_all_engine_barrier()

    def col(vn, i):
        return vsb[vn][:, i:i + 1]

    def lerp_into(dst, i, mu_name):
        d = jit3.tile([P, SUP], f32, tag="diff")
        nc.vector.tensor_sub(d, xext[:, i, 0:SUP], xext[:, i, 1:SUP + 1])
        nc.vector.scalar_tensor_tensor(
            out=dst, in0=d, scalar=col(mu_name, i), in1=xext[:, i, 1:SUP + 1],
            op0=OP.mult, op1=OP.add)

    for sp in range(NSUP):
        t0 = sp * SUP
        # ---- x load (t-layout halves) + PE transpose into xext ----
        for i in range(NCT):
            if sp == 0:
                nc.vector.memset(xext[:, i, 0:1], 0.0)
            else:
                nc.vector.tensor_copy(xext[:, i, 0:1], xext[:, i, SUP:SUP + 1])
        for g in range(SUP // P):
            for ih in range(2):
                xt = jit2.tile([P, C // 2], f32, tag="xtld")
                nc.sync.dma_start(
                    out=xt, in_=x_h[t0 + P * g:t0 + P * (g + 1),
                                    (C // 2) * ih:(C // 2) * (ih + 1)])
                for ii in range(NCT // 2):
                    i = (NCT // 2) * ih + ii
                    pt = ps_small.tile([P, P], f32, tag="ptr")
                    nc.tensor.transpose(pt, xt[:, P * ii:P * (ii + 1)], ident)
                    nc.scalar.copy(out=xext[:, i, 1 + P * g:1 + P * (g + 1)], in_=pt)
        # ---- lora A passes ----
        for n, mu in (("iclr_A", "mu_a"), ("decay_A", "mu_d"), ("gate_A", "mu_g")):
            pla = ps_lora.tile([LORA, SUP], f32, tag="pl")
            for i in range(NCT):
                laj = jit2.tile([P, LORA], f32r, tag="laj")
                nc.gpsimd.dma_start(out=laj, in_=la_h[n][P * i:P * (i + 1), :])
                xlo = jit2.tile([P, SUP], f32r, tag="xl")
                lerp_into(xlo, i, mu)
                nc.tensor.matmul(pla, laj, xlo,
                                 start=(i == 0), stop=(i == NCT - 1))
            nc.scalar.copy(out=la_out[n], in_=pla)
        # ---- lora B + activations ----
        for co in range(NCT):
            lb_i = jit2.tile([LORA, P], f32r, tag="lbi")
            nc.gpsimd.dma_start(out=lb_i, in_=lb_h["iclr_B"][:, P * co:P * (co + 1)])
            pib = ps_lora.tile([P, SUP], f32, tag="pl")
            nc.tensor.matmul(pib, lb_i, la_out["iclr_A"], start=True, stop=True)
            nc.scalar.activation(out=aT[:, co, :], in_=pib, func=AF.Sigmoid,
                                 bias=col("iclr_bias", co), scale=1.0)
            lb_d = jit2.tile([LORA, P], f32r, tag="lbi")
            nc.gpsimd.dma_start(out=lb_d, in_=lb_h["decay_B"][:, P * co:P * (co + 1)])
            pdb = ps_lora.tile([P, SUP], f32, tag="pl")
            nc.tensor.matmul(pdb, lb_d, la_out["decay_A"], start=True, stop=True)
            tmp = jit2.tile([P, SUP], f32, tag="acttmp")
            nc.scalar.activation(out=tmp, in_=pdb, func=AF.Tanh,
                                 bias=col("decay_bias", co), scale=1.0)
            nc.scalar.activation(out=tmp, in_=tmp, func=AF.Sigmoid)
            nc.scalar.activation(out=dT[:, co, :], in_=tmp, func=AF.Exp,
                                 scale=-DECAY_SCALE)
            lb_g = jit2.tile([LORA, P], f32r, tag="lbi")
            nc.gpsimd.dma_start(out=lb_g, in_=lb_h["gate_B"][:, P * co:P * (co + 1)])
            pgb = ps_lora.tile([P, SUP], f32, tag="pl")
            nc.tensor.matmul(pgb, lb_g, la_out["gate_A"], start=True, stop=True)
            nc.scalar.activation(out=gateT[:, co, :], in_=pgb, func=AF.Sigmoid)
        # ---- big projections (W_r resident; W_k/W_v streamed) ----
        for pn, mu in (("W_r", "mu_r"), ("W_k", "mu_k"), ("W_v", "mu_v")):
            for cop in range(4):
                pps = [ps_proj.tile([P, SUP], f32, tag="pp", name="pp")
                       for _ in range(2)]
                for i in range(NCT):
                    if pn == "W_r":
                        wtile = wr_sb[:, i, 256 * cop:256 * (cop + 1)]
                    else:
                        wtile = jit2.tile([P, 256], f32r, tag="wstr")
                        nc.gpsimd.dma_start(
                            out=wtile, in_=w_h[pn][P * i:P * (i + 1),
                                                   256 * cop:256 * (cop + 1)])
                    xl = jit2.tile([P, SUP], f32r, tag="xl")
                    lerp_into(xl, i, mu)
                    for cc in range(2):
                        nc.tensor.matmul(
                            pps[cc], wtile[:, P * cc:P * (cc + 1)], xl,
                            start=(i == 0), stop=(i == NCT - 1))
                for cc in range(2):
                    co = 2 * cop + cc
                    pslice = pps[cc]
                    if pn == "W_r":
                        nc.scalar.copy(out=rT[:, co, :], in_=pslice)
                    elif pn == "W_v":
                        nc.scalar.copy(out=vT[:, co, :], in_=pslice)
                    else:
                        nc.vector.tensor_scalar_mul(
                            out=knT[:, co, :], in0=pslice,
                            scalar1=col("removal_key_multiplier", co))
                        f = jit2.tile([P, SUP], f32, tag="fmix")
                        nc.vector.tensor_scalar(
                            out=f, in0=aT[:, co, :], scalar1=col("iclr_mix_amt", co),
                            scalar2=ommix[:, co:co + 1], op0=OP.mult, op1=OP.add)
                        nc.vector.tensor_mul(krT[:, co, :], pslice, f)
        # ---- removal-key norm + bonus pack ----
        pnb = ps_small.tile([16, 2 * SUP], f32, tag="ptr")
        for i in range(NCT):
            nsq = jit1.tile([P, 2 * SUP], f32r, tag="nsq")
            nc.vector.tensor_mul(nsq[:, 0:SUP], knT[:, i, :], knT[:, i, :])
            z2f = jit1.tile([P, SUP], f32, tag="z2f")
            nc.gpsimd.tensor_mul(z2f, rT[:, i, :], krT[:, i, :])
            nc.vector.tensor_scalar_mul(out=nsq[:, SUP:2 * SUP], in0=z2f,
                                        scalar1=col("bonus_multiplier", i))
            nc.tensor.matmul(pnb, inds[i], nsq, start=(i == 0), stop=(i == NCT - 1))
        nc.scalar.copy(out=nb, in_=pnb)
        nc.scalar.activation(out=nb[:, 0:SUP], in_=nb[:, 0:SUP], func=AF.Sqrt)
        nc.vector.tensor_scalar_max(out=nb[:, 0:SUP], in0=nb[:, 0:SUP],
                                    scalar1=NORM_EPS)
        nc.vector.reciprocal(out=nb[:, 0:SUP], in_=nb[:, 0:SUP])
        for i in range(NCT):
            rnb = jit1.tile([P, SUP], f32, tag="rnb")
            src = nb[2 * i:2 * i + 2, 0:SUP]
            nc.sync.dma_start(out=rnb, in_=bass.AP(
                tensor=src.tensor, offset=src.offset,
                ap=[src.ap[0], [0, 64], src.ap[1]]))
            nc.vector.tensor_mul(knT[:, i, :], knT[:, i, :], rnb)

        # ================= WKV chunks =================
        for ch in range(NCH):
            cs = ch * CH
            row = t0 + cs
            for i in range(NCT):
                nc.vector.memset(cext[:, i, 0:1], 1.0)
                nc.vector.tensor_tensor_scan(
                    out=cext[:, i, 1:CH + 1], data0=dT[:, i, cs:cs + CH],
                    data1=zeros, initial=1.0, op0=OP.mult, op1=OP.max)
                ci = jit2.tile([P, CH], f32, tag="ci")
                nc.vector.reciprocal(out=ci, in_=cext[:, i, 1:CH + 1])
                for srcT, dstt in ((knT[:, i, cs:cs + CH], knt_c[:, P * i:P * (i + 1)]),
                                   (krT[:, i, cs:cs + CH], krt_c[:, P * i:P * (i + 1)]),
                                   (vT[:, i, cs:cs + CH], vp_t[:, P * i:P * (i + 1)]),
                                   (ci, cit_c[:, P * i:P * (i + 1)])):
                    pt = ps_small.tile([P, P], f32, tag="ptr")
                    nc.tensor.transpose(pt, srcT, ident)
                    nc.scalar.copy(out=dstt, in_=pt)
                nc.vector.tensor_mul(wtf_c[:, i, :], knT[:, i, cs:cs + CH],
                                     cext[:, i, 0:CH])
                nc.vector.tensor_mul(wtf_c[:, i, :], wtf_c[:, i, :],
                                     aT[:, i, cs:cs + CH])
                nc.vector.tensor_copy(out=wtb_c[:, i, :], in_=wtf_c[:, i, :])
                nc.vector.tensor_mul(kntb_c[:, i, :], knT[:, i, cs:cs + CH], ci)
                nc.vector.tensor_mul(vld_c[:, i, :], vT[:, i, cs:cs + CH], ci)
            for i in range(NCT):
                nc.sync.dma_start(out=wtlo[:, i, :], in_=wtf_c[64:128, i, :])
                nc.sync.dma_start(out=rtlo[:, i, :], in_=rT[64:128, i, cs:cs + CH])
            nc.sync.dma_start(out=cl_al[:, 0, :], in_=cext[0:64, :, CH:CH + 1])
            nc.sync.dma_start(out=cl_al[:, 1, :], in_=cext[64:128, :, CH:CH + 1])
            nc.sync.dma_start(out=vp_h[row:row + CH, :], in_=vp_t)
            for h in range(H):
                i, hh = h // 2, h % 2
                ns = slice(64 * hh, 64 * (hh + 1))
                cn = slice(P * i + 64 * hh, P * i + 64 * (hh + 1))
                RT = rT[:, i, cs:cs + CH][ns, :]
                RT0 = rtlo[:, i, :] if hh else rT[0:64, i, cs:cs + CH]
                WT0 = wtlo[:, i, :] if hh else wtf_c[0:64, i, :]
                Svw, Stvw = S_st[:, i, hh, :], St_st[:, i, hh, :]
                cl_col = cl_al[:, hh, i:i + 1]
                pg = ps_wkv.tile([P, P], f32, tag="pwk")
                nc.tensor.matmul(pg, kntb_c[ns, i, :], wtb_c[ns, i, :],
                                 start=True, stop=True)
                Gu = pairp.tile([P, P], bf16, tag="Gu")
                nc.vector.tensor_mul(Gu, pg, mask_su)
                pa = ps_wkv.tile([P, P], f32, tag="pwk")
                nc.tensor.matmul(pa, vld_c[ns, i, :], wtf_c[ns, i, :],
                                 start=True, stop=True)
                Av = pairp.tile([P, P], f32, tag="Av")
                nc.vector.tensor_mul(Av, pa, mask_su)
                pp2 = ps_wkv.tile([P, P], f32, tag="pwk")
                nc.tensor.matmul(pp2, krT[:, i, cs:cs + CH][ns, :], RT,
                                 start=True, stop=True)
                Pm = pairp.tile([P, P], f32, tag="Pm")
                nc.vector.tensor_mul(Pm, pp2, mask_ui)
                pb = ps_wkv.tile([P, 64], f32, tag="pwk")
                nc.tensor.matmul(pb, WT0, Svw, start=True, stop=False)
                nc.tensor.matmul(pb, Av, krt_c[:, cn], start=False, stop=True)
                Bt = pairp.tile([P, 64], f32, tag="Bt")
                nc.scalar.copy(out=Bt, in_=pb)
                Xb = pairp.tile([P, 64], bf16, tag="Xb")
                nc.vector.tensor_copy(out=Xb, in_=pb)
                Um = None
                for it in range(NEUMANN_K):
                    px = ps_wkv.tile([P, 64], f32, tag="pwk")
                    nc.tensor.matmul(px, Gu, Xb, start=True, stop=True)
                    if it < NEUMANN_K - 1:
                        Xb = pairp.tile([P, 64], bf16, tag="Xb")
                        nc.vector.tensor_sub(Xb, Bt, px)
                    else:
                        Um = pairp.tile([P, 64], f32, tag="Um")
                        nc.vector.tensor_sub(Um, px, Bt)
                put = ps_small.tile([64, P], f32, tag="ptr", name="put")
                nc.tensor.transpose(put, Um, ident)
                Utf = pairp.tile([64, P], f32, tag="Ut")
                nc.scalar.copy(out=Utf, in_=put)
                pq = ps_wkv.tile([P, P], f32, tag="pwk")
                nc.tensor.matmul(pq, Utf, RT0, start=True, stop=True)
                Qm = pairp.tile([P, P], f32, tag="Qm")
                nc.vector.tensor_mul(Qm, pq, mask_ui)
                Vld = pairp.tile([P, 64], f32, tag="Vld")
                nc.vector.tensor_mul(Vld, vp_t[:, cn], cit_c[:, cn])
                Knl = pairp.tile([P, 64], f32, tag="Knl")
                nc.vector.tensor_mul(Knl, knt_c[:, cn], cit_c[:, cn])
                ctp = pairp.tile([P, 64], f32, tag="ctp")
                nc.vector.reciprocal(out=ctp, in_=cit_c[:, cn])
                po = ps_wkv.tile([P, 64], f32, tag="pwk")
                nc.tensor.matmul(po, RT0, Stvw, start=True, stop=False)
                nc.tensor.matmul(po, Pm, Vld, start=False, stop=False)
                nc.tensor.matmul(po, Qm, Knl, start=False, stop=True)
                nc.vector.tensor_mul(O_c[:, cn], po, ctp)
                pst = ps_small.tile([64, 64], f32, tag="ptr", name="pst")
                nc.tensor.matmul(pst, Vld, krt_c[:, cn], start=True, stop=False)
                nc.tensor.matmul(pst, Knl, Um, start=False, stop=True)
                nc.vector.tensor_add(Svw, Svw, pst)
                nc.vector.tensor_scalar_mul(out=Svw, in0=Svw, scalar1=cl_col)
                pstt = ps_small.tile([64, 64], f32, tag="ptr", name="pstt")
                nc.tensor.transpose(pstt, Svw, ident[0:64, 0:64])
                nc.scalar.copy(out=Stvw, in_=pstt)
            # ---- bonus + GroupNorm (t-layout) ----
            pbs = ps_small.tile([P, 16], f32, tag="ptr")
            nc.tensor.transpose(pbs, nb[:, SUP + cs:SUP + cs + CH],
                                ident[0:16, 0:16])
            nc.scalar.copy(out=bs_t, in_=pbs)
            for h in range(H):
                cn = slice(64 * h, 64 * (h + 1))
                nc.vector.scalar_tensor_tensor(
                    out=O_c[:, cn], in0=vp_t[:, cn], scalar=bs_t[:, h:h + 1],
                    in1=O_c[:, cn], op0=OP.mult, op1=OP.add)
                nc.vector.bn_stats(out=stats6[:, h, :], in_=O_c[:, cn])
                nc.vector.bn_aggr(out=mv2[:, h, :], in_=stats6[:, h, :])
            nc.scalar.activation(out=rstd, in_=mv2[:, :, 1], func=AF.Sqrt,
                                 bias=gn_eps)
            nc.vector.reciprocal(out=rstd, in_=rstd)
            for h in range(H):
                cn = slice(64 * h, 64 * (h + 1))
                nc.vector.tensor_scalar(
                    out=O_c[:, cn], in0=O_c[:, cn], scalar1=mv2[:, h, 0:1],
                    scalar2=rstd[:, h:h + 1], op0=OP.subtract, op1=OP.mult)
            # ---- ln/gate -> yT super staging (f32r) ----
            for i in range(NCT):
                pzt = ps_small.tile([P, P], f32, tag="ptr")
                nc.tensor.transpose(pzt, O_c[:, P * i:P * (i + 1)], ident)
                yt1 = jit2.tile([P, P], f32, tag="yt1")
                nc.vector.tensor_scalar(
                    out=yt1, in0=pzt, scalar1=col("ln_w", i),
                    scalar2=col("ln_b", i), op0=OP.mult, op1=OP.add)
                nc.vector.tensor_mul(yTs[:, i, cs:cs + CH], yt1,
                                     gateT[:, i, cs:cs + CH])
        # ---- W_o (per super, streamed weights) ----
        for half in range(2):
            pys = [ps_proj.tile([P, C // 2], f32, tag="pp", name="pyo")
                   for _ in range(NCH)]
            for i in range(NCT):
                wo = jit2.tile([P, C // 2], f32r, tag="wstr")
                nc.gpsimd.dma_start(
                    out=wo, in_=w_h["W_o"][P * i:P * (i + 1),
                                           (C // 2) * half:(C // 2) * (half + 1)])
                for chh in range(NCH):
                    nc.tensor.matmul(pys[chh], yTs[:, i, CH * chh:CH * (chh + 1)],
                                     wo, start=(i == 0), stop=(i == NCT - 1))
            for chh in range(NCH):
                yst = jit2.tile([P, C // 2], f32, tag="yst")
                nc.scalar.copy(out=yst, in_=pys[chh])
                nc.sync.dma_start(
                    out=y_h[t0 + CH * chh:t0 + CH * (chh + 1),
                            (C // 2) * half:(C // 2) * (half + 1)],
                    in_=yst)
    ctx.close()


def kernel(**inputs):
    from concourse.bass_utils import run_bass_kernel_spmd
    if "nc" not in _CACHE:
        _CACHE["nc"] = _build()
    nc = _CACHE["nc"]
    x = np.ascontiguousarray(np.asarray(inputs["x"], dtype=np.float32))
    shared = {n: np.ascontiguousarray(np.asarray(inputs[n], dtype=np.float32))
              for n in MAT_NAMES + VEC_NAMES}
    in_maps = [dict(shared, x=np.ascontiguousarray(x[b])) for b in range(B)]
    res = run_bass_kernel_spmd(nc, in_maps, core_ids=list(range(B)))
    y = np.stack([res.results[b]["y"] for b in range(B)])
    vp = np.stack([res.results[b]["vp"] for b in range(B)])
    return y, vp



# revision 46
# speedup vs baseline: 1.4474x; 1.4474x over previous
"""RWKV-7 TimeMix kernel for 8 Trainium2 NeuronCores.

Sharding: data-parallel over B (8 batches -> 8 cores). Each core runs the
full per-batch module: time-shift lerps, r/k/v projections, LoRA branches
(decay/iclr/gate), the WKV state recurrence (chunked UT-transform with a
truncated-Neumann intra-chunk solve), bonus, GroupNorm, output gate, W_o.

Chunked WKV math per head (chunk L=128, state S[i,j], c = in-chunk cumprod
of the decay d):
  Wt[t] = a_t*kn_t*c_{t-1}      Kn[s] = kn_s/c_s       Vs[s] = v_s/c_s
  G  = triu(Kn Wt^T, 1)   Av = triu(Vs Wt^T, 1)   P = triu(kr r^T, 0)
  B  = Wt S0 + Av^T kr
  (I+G^T) U = B  solved by K Neumann steps  X <- B - G^T X;  Um := -U
  Qm = triu(Um r^T, 0)
  O[t] = c_t * (r S0^T + P^T Vs + Qm^T Kn)
  S   <- diag(c_L) (S + Vs^T kr + Kn^T Um)

v1 perf notes: everything matmul-heavy runs bf16 (1 cycle/row on the PE at
any clock state; fp32/f32r run 2-4x slower at the cold 1.2GHz p-state this
kernel measures at). All four big weights + LoRA mats live SBUF-resident in
bf16 (loaded once, ~17MB HBM instead of ~100MB streamed). Neumann applies
use an identity-inject (px = I@B + (-G)@X) so the per-step combine is a
scalar-engine copy instead of a vector sub; masks/elementwise are spread
across Vector/GpSimd/Scalar to balance engine occupancy; stages are emitted
head-major-interleaved so the 16 independent per-head chains pipeline.
"""
import numpy as np

B, T, C, H, N = 8, 2048, 1024, 16, 64
LORA = 64
P = 128
NCT = C // P          # 8 channel tiles
CH = 128              # WKV chunk
SUP = 256             # projection super-chunk
NSUP = T // SUP       # 8
NCH = SUP // CH       # 2
NEUMANN_K = 6
DECAY_SCALE = float(np.exp(-0.5))
GN_EPS = 1e-5 * H
NORM_EPS = 1e-12

VEC_NAMES = ["mu_r", "mu_k", "mu_v", "mu_g", "mu_a", "mu_d",
             "decay_bias", "iclr_bias", "removal_key_multiplier",
             "iclr_mix_amt", "bonus_multiplier", "ln_w", "ln_b"]
MAT_NAMES = ["W_r", "W_k", "W_v", "W_o", "decay_A", "iclr_A", "gate_A",
             "decay_B", "iclr_B", "gate_B"]

_CACHE = {}


def _build():
    import concourse.bass as bass  # noqa: F401
    from concourse import bacc, mybir
    import concourse.tile as tile

    f32 = mybir.dt.float32
    nc = bacc.Bacc("TRN2", target_bir_lowering=False, debug=False, num_devices=B)
    x_h = nc.dram_tensor("x", [T, C], f32, kind="ExternalInput")
    w_h = {n: nc.dram_tensor(n, [C, C], f32, kind="ExternalInput")
           for n in ("W_r", "W_k", "W_v", "W_o")}
    la_h = {n: nc.dram_tensor(n, [C, LORA], f32, kind="ExternalInput")
            for n in ("decay_A", "iclr_A", "gate_A")}
    lb_h = {n: nc.dram_tensor(n, [LORA, C], f32, kind="ExternalInput")
            for n in ("decay_B", "iclr_B", "gate_B")}
    vec_h = {n: nc.dram_tensor(n, [C], f32, kind="ExternalInput") for n in VEC_NAMES}
    y_h = nc.dram_tensor("y", [T, C], f32, kind="ExternalOutput")
    vp_h = nc.dram_tensor("vp", [T, C], f32, kind="ExternalOutput")
    with tile.TileContext(nc) as tc:
        _emit(nc, tc, x_h, w_h, la_h, lb_h, vec_h, y_h, vp_h)
    nc.finalize()
    return nc


def _emit(nc, tc, x_h, w_h, la_h, lb_h, vec_h, y_h, vp_h):
    import concourse.bass as bass
    from concourse import mybir
    from concourse.masks import make_identity
    from contextlib import ExitStack

    f32 = mybir.dt.float32
    f32r = mybir.dt.float32r
    bf16 = mybir.dt.bfloat16
    f16 = mybir.dt.float16
    AF = mybir.ActivationFunctionType
    OP = mybir.AluOpType

    ctx = ExitStack()
    const = ctx.enter_context(tc.tile_pool(name="const", bufs=1))
    supp = ctx.enter_context(tc.tile_pool(name="supp", bufs=1))
    chk2 = ctx.enter_context(tc.tile_pool(name="chk2", bufs=2))
    chk1 = ctx.enter_context(tc.tile_pool(name="chk1", bufs=1))
    jit1 = ctx.enter_context(tc.tile_pool(name="jit1", bufs=1))
    jit2 = ctx.enter_context(tc.tile_pool(name="jit2", bufs=2))
    jit3 = ctx.enter_context(tc.tile_pool(name="jit3", bufs=3))
    ps_proj = ctx.enter_context(tc.tile_pool(name="ps_proj", bufs=2, space="PSUM"))
    ps_lora = ps_proj          # all big [P,<=512] f32 accumulators share tag "pp"
    ps_g = ctx.enter_context(tc.tile_pool(name="ps_g", bufs=2, space="PSUM"))
    ps_x = ctx.enter_context(tc.tile_pool(name="ps_x", bufs=2, space="PSUM"))
    ps_s = ps_x                # [64,64] state tiles share tag "px"
    ps_b = ctx.enter_context(tc.tile_pool(name="ps_b", bufs=2, space="PSUM"))

    # ---------------- resident weights (bf16) ----------------
    wsb = {n: const.tile([P, NCT, C], f16, tag=f"w_{n}", name=f"w_{n}")
           for n in ("W_r", "W_k", "W_v")}
    la_sb = {n: const.tile([P, NCT, LORA], f16, tag=f"la_{n}", name=f"la_{n}")
             for n in ("decay_A", "iclr_A", "gate_A")}
    lb_sb = {n: const.tile([LORA, C], f16, tag=f"lb_{n}", name=f"lb_{n}")
             for n in ("decay_B", "iclr_B", "gate_B")}
    for n in ("W_r", "W_k", "W_v"):
        for i in range(NCT):
            for hf in range(4):
                wj = jit2.tile([P, C // 4], f32, tag="wldj", name="wldj")
                nc.gpsimd.dma_start(
                    out=wj, in_=w_h[n][P * i:P * (i + 1),
                                       (C // 4) * hf:(C // 4) * (hf + 1)])
                dst = wsb[n][:, i, (C // 4) * hf:(C // 4) * (hf + 1)]
                if hf % 2 == 0:
                    nc.scalar.copy(out=dst, in_=wj)
                else:
                    nc.vector.tensor_copy(out=dst, in_=wj)
    for n in ("decay_A", "iclr_A", "gate_A"):
        for i in range(NCT):
            lj = jit2.tile([P, LORA], f32, tag="lldj", name="lldj")
            nc.gpsimd.dma_start(out=lj, in_=la_h[n][P * i:P * (i + 1), :])
            nc.scalar.copy(out=la_sb[n][:, i, :], in_=lj)
    for n in ("decay_B", "iclr_B", "gate_B"):
        for hf in range(4):
            bj = jit2.tile([LORA, C // 4], f32, tag="bldj", name="bldj")
            nc.gpsimd.dma_start(
                out=bj, in_=lb_h[n][:, (C // 4) * hf:(C // 4) * (hf + 1)])
            nc.vector.tensor_copy(
                out=lb_sb[n][:, (C // 4) * hf:(C // 4) * (hf + 1)], in_=bj)

    # ---------------- static constants ----------------
    vsb = {}
    for n in VEC_NAMES:
        t = const.tile([P, NCT], f32, tag=f"v_{n}", name=f"v_{n}")
        src = vec_h[n][:]
        nc.sync.dma_start(out=t, in_=bass.AP(
            tensor=src.tensor, offset=src.offset, ap=[[1, P], [P, NCT]]))
        vsb[n] = t
    ommix = const.tile([P, NCT], f32, tag="v_ommix")
    nc.vector.tensor_scalar(out=ommix, in0=vsb["iclr_mix_amt"], scalar1=-1.0,
                            scalar2=1.0, op0=OP.mult, op1=OP.add)
    ident = const.tile([P, P], f32, tag="ident")
    make_identity(nc, ident)
    ident_b = const.tile([P, P], bf16, tag="ident_b")
    make_identity(nc, ident_b)
    ident_h = const.tile([P, P], f16, tag="ident_h")
    make_identity(nc, ident_h)
    # masks (bf16): value kept where predicate true, fill elsewhere
    mask_su = const.tile([P, P], bf16, tag="mask_su")    # +1 where s < t
    nc.gpsimd.memset(mask_su, 1.0)
    nc.gpsimd.affine_select(out=mask_su, in_=mask_su, compare_op=OP.is_gt,
                            fill=0.0, base=0, channel_multiplier=-1,
                            pattern=[[1, P]])
    mask_sun = const.tile([P, P], bf16, tag="mask_sun")  # -1 where s < t
    nc.gpsimd.memset(mask_sun, -1.0)
    nc.gpsimd.affine_select(out=mask_sun, in_=mask_sun, compare_op=OP.is_gt,
                            fill=0.0, base=0, channel_multiplier=-1,
                            pattern=[[1, P]])
    mask_ui = const.tile([P, P], bf16, tag="mask_ui")    # +1 where s <= t
    nc.gpsimd.memset(mask_ui, 1.0)
    nc.gpsimd.affine_select(out=mask_ui, in_=mask_ui, compare_op=OP.is_ge,
                            fill=0.0, base=0, channel_multiplier=-1,
                            pattern=[[1, P]])
    inds = []
    for i in range(NCT):
        indf = const.tile([P, 16], f32, tag=f"indf{i}", name=f"indf{i}")
        nc.vector.memset(indf, 0.0)
        nc.vector.memset(indf[0:64, 2 * i:2 * i + 1], 1.0)
        nc.vector.memset(indf[64:128, 2 * i + 1:2 * i + 2], 1.0)
        indr = const.tile([P, 16], f32r, tag=f"indr{i}", name=f"indr{i}")
        nc.scalar.copy(out=indr, in_=indf)
        inds.append(indr)
    zeros = const.tile([P, CH], f32, tag="zeros")
    nc.vector.memset(zeros, 0.0)
    gn_eps = const.tile([P, 1], f32, tag="gn_eps")
    nc.vector.memset(gn_eps, GN_EPS)
    S_st = const.tile([64, NCT, 2, 64], f32, tag="S_st")
    Sb_st = const.tile([64, NCT, 2, 64], bf16, tag="Sb_st")
    Stb_st = const.tile([64, NCT, 2, 64], f16, tag="Stb_st")
    nc.vector.memset(S_st, 0.0)
    nc.vector.memset(Sb_st, 0.0)
    nc.vector.memset(Stb_st, 0.0)

    # ------------- per-super persistents -------------
    xext = supp.tile([P, NCT, SUP + 1], f16, tag="xext")
    xlp1 = supp.tile([P, NCT, SUP], f16, tag="xlp1")
    rT = supp.tile([P, NCT, SUP], f16, tag="rT")
    knT = supp.tile([P, NCT, SUP], f16, tag="knT")
    krT = supp.tile([P, NCT, SUP], f16, tag="krT")
    vT = supp.tile([P, NCT, SUP], bf16, tag="vT")
    aT = supp.tile([P, NCT, SUP], f16, tag="aT")
    dT = supp.tile([P, NCT, SUP], f32, tag="dT")
    gateT = supp.tile([P, NCT, SUP], f16, tag="gateT")
    yTs = supp.tile([P, NCT, SUP], f16, tag="yTs")
    nbr = supp.tile([16, SUP], f32, tag="nbr")
    la_out = {n: supp.tile([LORA, SUP], f16, tag=f"lo_{n}", name=f"lo_{n}")
              for n in ("decay_A", "iclr_A", "gate_A")}
    nb = supp.tile([16, 2 * SUP], f32, tag="nb")
    # ------------- per-chunk persistents -------------
    # double-buffered (written by chunk pre-stage, consumed through the
    # head stages -> lets chunk k+1 preprocessing overlap chunk k solve)
    # single-buffered late-stage tiles
    Bt_all = chk1.tile([P, H, 64], f16, tag="Bt_all")
    X_all = chk1.tile([P, H, 64], f16, tag="X_all")
    Um_all = chk1.tile([P, H, 64], bf16, tag="Um_all")
    Utf_all = chk1.tile([64, H, P], f16, tag="Utf_all")
    ct_t = chk1.tile([P, C], f32, tag="ct_t")
    vp_t = chk1.tile([P, C], f32, tag="vp_t")
    O_c = chk1.tile([P, C], f32, tag="O_c")
    bs_t = chk1.tile([P, 16], f32, tag="bs_t")
    stats6 = chk1.tile([P, 16, 6], f32, tag="stats6")
    mv2 = chk1.tile([P, 16, 2], f32, tag="mv2")
    sstd = chk1.tile([P, 16], f32, tag="sstd")
    rstd = chk1.tile([P, 16], f32, tag="rstd")
    cl_al = chk1.tile([64, 2, NCT], f32, tag="cl_al")

    tc.strict_bb_all_engine_barrier()

    def col(vn, i):
        return vsb[vn][:, i:i + 1]

    for sp in range(NSUP):
        t0 = sp * SUP
        # ---- x load (t-layout halves) + bf16 cast + PE transpose ----
        for i in range(NCT):
            if sp == 0:
                nc.vector.memset(xext[:, i, 0:1], 0.0)
            else:
                nc.vector.tensor_copy(xext[:, i, 0:1], xext[:, i, SUP:SUP + 1])
        for g in range(SUP // P):
            for ih in range(2):
                xt = jit1.tile([P, C // 2], f32, tag="xtld")
                nc.sync.dma_start(
                    out=xt, in_=x_h[t0 + P * g:t0 + P * (g + 1),
                                    (C // 2) * ih:(C // 2) * (ih + 1)])
                for iq in range(2):
                    xtb = jit2.tile([P, C // 4], f16, tag="xtbc")
                    nc.vector.tensor_copy(
                        out=xtb, in_=xt[:, (C // 4) * iq:(C // 4) * (iq + 1)])
                    for ij in range(NCT // 4):
                        ii = (NCT // 4) * iq + ij
                        i = (NCT // 2) * ih + ii
                        pt = ps_b.tile([P, P], f16, tag="ptb", name="ptx")
                        nc.tensor.transpose(pt, xtb[:, P * ij:P * (ij + 1)],
                                            ident_h)
                        nc.scalar.copy(
                            out=xext[:, i, 1 + P * g:1 + P * (g + 1)], in_=pt)
        # ---- lora A passes (resident bf16 weights) ----
        for n, mu in (("iclr_A", "mu_a"), ("decay_A", "mu_d"),
                      ("gate_A", "mu_g")):
            pla = ps_lora.tile([LORA, SUP], f32, tag="pp", name="pla")
            for i in range(NCT):
                dx = jit3.tile([P, SUP], f16, tag="dxj")
                nc.vector.tensor_sub(dx, xext[:, i, 0:SUP],
                                     xext[:, i, 1:SUP + 1])
                xlo = jit3.tile([P, SUP], f16, tag="xlo")
                nc.vector.scalar_tensor_tensor(
                    out=xlo, in0=dx, scalar=col(mu, i),
                    in1=xext[:, i, 1:SUP + 1], op0=OP.mult, op1=OP.add)
                nc.tensor.matmul(pla, la_sb[n][:, i, :], xlo,
                                 start=(i == 0), stop=(i == NCT - 1))
            nc.scalar.copy(out=la_out[n], in_=pla)
        # ---- lora B + activations (function-major to avoid table thrash) --
        pib_l, pgb_l, pdb_l = [], [], []
        for co in range(NCT):
            pib = ps_lora.tile([P, SUP], f32, tag="pp", name="pib")
            nc.tensor.matmul(pib, lb_sb["iclr_B"][:, P * co:P * (co + 1)],
                             la_out["iclr_A"], start=True, stop=True)
            nc.scalar.activation(out=aT[:, co, :], in_=pib, func=AF.Sigmoid,
                                 bias=col("iclr_bias", co), scale=1.0)
            pgb = ps_lora.tile([P, SUP], f32, tag="pp", name="pgb")
            nc.tensor.matmul(pgb, lb_sb["gate_B"][:, P * co:P * (co + 1)],
                             la_out["gate_A"], start=True, stop=True)
            nc.scalar.activation(out=gateT[:, co, :], in_=pgb, func=AF.Sigmoid)
            pdb = ps_lora.tile([P, SUP], f32, tag="pp", name="pdb")
            nc.tensor.matmul(pdb, lb_sb["decay_B"][:, P * co:P * (co + 1)],
                             la_out["decay_A"], start=True, stop=True)
            nc.scalar.activation(out=dT[:, co, :], in_=pdb, func=AF.Tanh,
                                 bias=col("decay_bias", co), scale=1.0)
        for co in range(NCT):
            nc.scalar.activation(out=dT[:, co, :], in_=dT[:, co, :],
                                 func=AF.Sigmoid)
        for co in range(NCT):
            nc.scalar.activation(out=dT[:, co, :], in_=dT[:, co, :],
                                 func=AF.Exp, scale=-DECAY_SCALE)
        # ---- big projections (all-resident bf16 weights) ----
        for pn, mu in (("W_r", "mu_r"), ("W_k", "mu_k"), ("W_v", "mu_v")):
            for i in range(NCT):
                dx = jit3.tile([P, SUP], f16, tag="dxj")
                nc.vector.tensor_sub(dx, xext[:, i, 0:SUP],
                                     xext[:, i, 1:SUP + 1])
                nc.vector.scalar_tensor_tensor(
                    out=xlp1[:, i, :], in0=dx, scalar=col(mu, i),
                    in1=xext[:, i, 1:SUP + 1], op0=OP.mult, op1=OP.add)
            for cop in range(4):
                pps = [ps_proj.tile([P, SUP], f32, tag="pp", name="pp")
                       for _ in range(2)]
                for i in range(NCT):
                    for cc in range(2):
                        cbase = 256 * cop + P * cc
                        nc.tensor.matmul(
                            pps[cc], wsb[pn][:, i, cbase:cbase + P],
                            xlp1[:, i, :],
                            start=(i == 0), stop=(i == NCT - 1))
                for cc in range(2):
                    co = 2 * cop + cc
                    pslice = pps[cc]
                    if pn == "W_r":
                        nc.scalar.copy(out=rT[:, co, :], in_=pslice)
                    elif pn == "W_v":
                        nc.scalar.copy(out=vT[:, co, :], in_=pslice)
                    else:
                        nc.vector.tensor_scalar_mul(
                            out=knT[:, co, :], in0=pslice,
                            scalar1=col("removal_key_multiplier", co))
                        f = jit1.tile([P, SUP], f16, tag="fmix")
                        nc.vector.tensor_scalar(
                            out=f, in0=aT[:, co, :],
                            scalar1=col("iclr_mix_amt", co),
                            scalar2=ommix[:, co:co + 1],
                            op0=OP.mult, op1=OP.add)
                        nc.vector.tensor_mul(krT[:, co, :], pslice, f)
        # ---- removal-key norm + bonus pack ----
        pnb = ps_lora.tile([16, 2 * SUP], f32, tag="pp", name="pnb")
        for i in range(NCT):
            nsq = jit1.tile([P, 2 * SUP], f32r, tag="nsq")
            nc.vector.tensor_mul(nsq[:, 0:SUP], knT[:, i, :], knT[:, i, :])
            nc.vector.tensor_mul(nsq[:, SUP:2 * SUP], rT[:, i, :],
                                 krT[:, i, :])
            nc.vector.tensor_scalar_mul(out=nsq[:, SUP:2 * SUP],
                                        in0=nsq[:, SUP:2 * SUP],
                                        scalar1=col("bonus_multiplier", i))
            nc.tensor.matmul(pnb, inds[i], nsq, start=(i == 0),
                             stop=(i == NCT - 1))
        nc.scalar.copy(out=nb, in_=pnb)
        nc.scalar.activation(out=nb[:, 0:SUP], in_=nb[:, 0:SUP], func=AF.Sqrt)
        nc.vector.tensor_scalar_max(out=nb[:, 0:SUP], in0=nb[:, 0:SUP],
                                    scalar1=NORM_EPS)
        nc.vector.reciprocal_approx_fast(out=nbr, in_=nb[:, 0:SUP])
        for i in range(NCT):
            rnb = jit1.tile([P, SUP], f32, tag="rnb")
            src = nbr[2 * i:2 * i + 2, :]
            nc.sync.dma_start(out=rnb, in_=bass.AP(
                tensor=src.tensor, offset=src.offset,
                ap=[src.ap[0], [0, 64], src.ap[1]]))
            nc.vector.tensor_mul(knT[:, i, :], knT[:, i, :], rnb)

        # ================= WKV chunks =================
        for ch in range(NCH):
            cs = ch * CH
            row = t0 + cs
            # rotating double-buffered chunk tiles
            cext = chk2.tile([P, NCT, CH + 1], f32, tag="cext")
            wtb_c = chk2.tile([P, NCT, CH], bf16, tag="wtb_c")
            kntb_c = chk2.tile([P, NCT, CH], bf16, tag="kntb_c")
            vldb_c = chk2.tile([P, NCT, CH], bf16, tag="vldb_c")
            krt_c = chk1.tile([P, C], bf16, tag="krt_c")
            Vld_t = chk2.tile([P, C], bf16, tag="Vld_t")
            Knl_t = chk2.tile([P, C], bf16, tag="Knl_t")
            Gun_all = chk1.tile([P, H, P], f16, tag="Gun_all")
            Av_all = chk1.tile([P, H, P], bf16, tag="Av_all")
            wtlo = chk1.tile([64, NCT, CH], bf16, tag="wtlo")
            rtlo = chk1.tile([64, NCT, CH], f16, tag="rtlo")
            # ---- per-i preprocessing ----
            for i in range(NCT):
                nc.vector.memset(cext[:, i, 0:1], 1.0)
                nc.vector.tensor_tensor_scan(
                    out=cext[:, i, 1:CH + 1], data0=dT[:, i, cs:cs + CH],
                    data1=zeros, initial=1.0, op0=OP.mult, op1=OP.max)
                ci = jit2.tile([P, CH], f32, tag="ci")
                nc.vector.reciprocal_approx_fast(out=ci, in_=cext[:, i, 1:CH + 1])
                tmpw = jit2.tile([P, CH], f32, tag="tmpw")
                nc.gpsimd.tensor_mul(tmpw, knT[:, i, cs:cs + CH],
                                     cext[:, i, 0:CH])
                nc.gpsimd.tensor_mul(wtb_c[:, i, :], tmpw, aT[:, i, cs:cs + CH])
                nc.gpsimd.tensor_mul(kntb_c[:, i, :], knT[:, i, cs:cs + CH], ci)
                nc.gpsimd.tensor_mul(vldb_c[:, i, :], vT[:, i, cs:cs + CH], ci)
                # transposes into t-major
                ptk = ps_b.tile([P, P], f16, tag="ptb", name="ptk")
                nc.tensor.transpose(ptk, krT[:, i, cs:cs + CH], ident_h)
                nc.scalar.copy(out=krt_c[:, P * i:P * (i + 1)], in_=ptk)
                ptv = ps_b.tile([P, P], bf16, tag="ptb")
                nc.tensor.transpose(ptv, vldb_c[:, i, :], ident_b)
                nc.scalar.copy(out=Vld_t[:, P * i:P * (i + 1)], in_=ptv)
                ptn = ps_b.tile([P, P], bf16, tag="ptb")
                nc.tensor.transpose(ptn, kntb_c[:, i, :], ident_b)
                nc.scalar.copy(out=Knl_t[:, P * i:P * (i + 1)], in_=ptn)
                ptp = ps_b.tile([P, P], bf16, tag="ptb")
                nc.tensor.transpose(ptp, vT[:, i, cs:cs + CH], ident_b)
                nc.scalar.copy(out=vp_t[:, P * i:P * (i + 1)], in_=ptp)
                ptc = ps_g.tile([P, P], f32, tag="pg", name="ptf")
                nc.tensor.transpose(ptc, cext[:, i, 1:CH + 1], ident)
                nc.vector.tensor_copy(out=ct_t[:, P * i:P * (i + 1)], in_=ptc)
            for i in range(NCT):
                nc.sync.dma_start(out=wtlo[:, i, :], in_=wtb_c[64:128, i, :])
                nc.sync.dma_start(out=rtlo[:, i, :], in_=rT[64:128, i, cs:cs + CH])
            nc.sync.dma_start(out=cl_al[:, 0, :], in_=cext[0:64, :, CH:CH + 1])
            nc.sync.dma_start(out=cl_al[:, 1, :], in_=cext[64:128, :, CH:CH + 1])
            nc.sync.dma_start(out=vp_h[row:row + CH, :], in_=vp_t)

            def hparts(h):
                i, hh = h // 2, h % 2
                ns = slice(64 * hh, 64 * (hh + 1))
                cn = slice(P * i + 64 * hh, P * i + 64 * (hh + 1))
                return i, hh, ns, cn

            # ---- stage: G / Av generation ----
            for h in range(H):
                i, hh, ns, cn = hparts(h)
                pg = ps_g.tile([P, P], f32, tag="pg")
                nc.tensor.matmul(pg, kntb_c[ns, i, :], wtb_c[ns, i, :],
                                 start=True, stop=True)
                nc.vector.tensor_mul(Gun_all[:, h, :], pg, mask_sun)
                pa = ps_g.tile([P, P], f32, tag="pg")
                nc.tensor.matmul(pa, vldb_c[ns, i, :], wtb_c[ns, i, :],
                                 start=True, stop=True)
                nc.vector.tensor_mul(Av_all[:, h, :], pa, mask_su)
            # ---- stage: B = Wt S0 + Av^T kr ----
            for h in range(H):
                i, hh, ns, cn = hparts(h)
                WT0 = wtlo[:, i, :] if hh else wtb_c[0:64, i, :]
                pb = ps_x.tile([P, 64], f32, tag="px")
                nc.tensor.matmul(pb, WT0, Sb_st[:, i, hh, :],
                                 start=True, stop=False)
                nc.tensor.matmul(pb, Av_all[:, h, :], krt_c[:, cn],
                                 start=False, stop=True)
                nc.scalar.copy(out=Bt_all[:, h, :], in_=pb)
            # ---- stage: Neumann iterations (ident-inject form) ----
            for it in range(NEUMANN_K):
                for h in range(H):
                    i, hh, ns, cn = hparts(h)
                    xin = Bt_all[:, h, :] if it == 0 else X_all[:, h, :]
                    px = ps_x.tile([P, 64], f32, tag="px")
                    nc.tensor.matmul(px, ident_h, Bt_all[:, h, :],
                                     start=True, stop=False)
                    nc.tensor.matmul(px, Gun_all[:, h, :], xin,
                                     start=False, stop=True)
                    if it < NEUMANN_K - 1:
                        nc.scalar.copy(out=X_all[:, h, :], in_=px)
                    else:
                        nc.scalar.activation(out=Um_all[:, h, :], in_=px,
                                             func=AF.Copy, scale=-1.0)
            # ---- stage: U^T + Q matrix ----
            for h in range(H):
                i, hh, ns, cn = hparts(h)
                put = ps_b.tile([64, P], bf16, tag="ptb", name="put")
                nc.tensor.transpose(put, Um_all[:, h, :], ident_b)
                nc.scalar.copy(out=Utf_all[:, h, :], in_=put)
            # ---- stage: P/Q matrices + output O ----
            for h in range(H):
                i, hh, ns, cn = hparts(h)
                RT = rT[:, i, cs:cs + CH][ns, :]
                RT0 = rtlo[:, i, :] if hh else rT[0:64, i, cs:cs + CH]
                pp2 = ps_g.tile([P, P], f32, tag="pg")
                nc.tensor.matmul(pp2, krT[:, i, cs:cs + CH][ns, :], RT,
                                 start=True, stop=True)
                Pm = jit3.tile([P, P], bf16, tag="Pmj")
                nc.vector.tensor_mul(Pm, pp2, mask_ui)
                pq = ps_g.tile([P, P], f32, tag="pg")
                nc.tensor.matmul(pq, Utf_all[:, h, :], RT0,
                                 start=True, stop=True)
                Qm = jit3.tile([P, P], bf16, tag="Qmj")
                nc.vector.tensor_mul(Qm, pq, mask_ui)
                po = ps_x.tile([P, 64], f32, tag="px")
                nc.tensor.matmul(po, RT0, Stb_st[:, i, hh, :],
                                 start=True, stop=False)
                nc.tensor.matmul(po, Pm, Vld_t[:, cn],
                                 start=False, stop=False)
                nc.tensor.matmul(po, Qm, Knl_t[:, cn],
                                 start=False, stop=True)
                nc.vector.tensor_mul(O_c[:, cn], po, ct_t[:, cn])
            # ---- stage: state update ----
            for h in range(H):
                i, hh, ns, cn = hparts(h)
                Svw = S_st[:, i, hh, :]
                pst = ps_s.tile([64, 64], f32, tag="px", name="pst")
                nc.tensor.matmul(pst, Vld_t[:, cn], krt_c[:, cn],
                                 start=True, stop=False)
                nc.tensor.matmul(pst, Knl_t[:, cn], Um_all[:, h, :],
                                 start=False, stop=True)
                nc.vector.tensor_add(Svw, Svw, pst)
                nc.vector.tensor_scalar_mul(out=Svw, in0=Svw,
                                            scalar1=cl_al[:, hh, i:i + 1])
                nc.gpsimd.tensor_copy(out=Sb_st[:, i, hh, :], in_=Svw)
                pstt = ps_s.tile([64, 64], f32, tag="px", name="pstt")
                nc.tensor.transpose(pstt, Svw, ident[0:64, 0:64])
                nc.scalar.copy(out=Stb_st[:, i, hh, :], in_=pstt)
            # ---- bonus + GroupNorm (t-layout) ----
            pbs = ps_g.tile([P, 16], f32, tag="pg", name="pbs")
            nc.tensor.transpose(pbs, nb[:, SUP + cs:SUP + cs + CH],
                                ident[0:16, 0:16])
            nc.scalar.copy(out=bs_t, in_=pbs)
            for h in range(H):
                cn = slice(64 * h, 64 * (h + 1))
                nc.vector.scalar_tensor_tensor(
                    out=O_c[:, cn], in0=vp_t[:, cn], scalar=bs_t[:, h:h + 1],
                    in1=O_c[:, cn], op0=OP.mult, op1=OP.add)
                nc.vector.bn_stats(out=stats6[:, h, :], in_=O_c[:, cn])
                nc.vector.bn_aggr(out=mv2[:, h, :], in_=stats6[:, h, :])
            nc.scalar.activation(out=sstd, in_=mv2[:, :, 1], func=AF.Sqrt,
                                 bias=gn_eps)
            nc.vector.reciprocal_approx_fast(out=rstd, in_=sstd)
            for h in range(H):
                cn = slice(64 * h, 64 * (h + 1))
                nc.vector.tensor_scalar(
                    out=O_c[:, cn], in0=O_c[:, cn],
                    scalar1=mv2[:, h, 0:1], scalar2=rstd[:, h:h + 1],
                    op0=OP.subtract, op1=OP.mult)
            # ---- ln/gate -> yT super staging (bf16) ----
            for i in range(NCT):
                pzt = ps_g.tile([P, P], f32, tag="pg", name="ptf")
                nc.tensor.transpose(pzt, O_c[:, P * i:P * (i + 1)], ident)
                yt1 = jit2.tile([P, P], f32, tag="yt1")
                nc.vector.tensor_scalar(
                    out=yt1, in0=pzt, scalar1=col("ln_w", i),
                    scalar2=col("ln_b", i), op0=OP.mult, op1=OP.add)
                nc.gpsimd.tensor_mul(yTs[:, i, cs:cs + CH], yt1,
                                     gateT[:, i, cs:cs + CH])
        # ---- W_o (per super, streamed f32r moving operand) ----
        for half in range(2):
            pys = [ps_proj.tile([P, C // 2], f32, tag="pp", name="pyo")
                   for _ in range(NCH)]
            for i in range(NCT):
                wo = jit1.tile([P, C // 2], f32, tag="wstr")
                nc.gpsimd.dma_start(
                    out=wo, in_=w_h["W_o"][P * i:P * (i + 1),
                                           (C // 2) * half:(C // 2) * (half + 1)])
                wob = jit2.tile([P, C // 2], f16, tag="wob")
                nc.gpsimd.tensor_copy(out=wob, in_=wo)
                for chh in range(NCH):
                    nc.tensor.matmul(
                        pys[chh], yTs[:, i, CH * chh:CH * (chh + 1)], wob,
                        start=(i == 0), stop=(i == NCT - 1))
            for chh in range(NCH):
                yst = jit1.tile([P, C // 2], f32, tag="yst")
                nc.scalar.copy(out=yst, in_=pys[chh])
                nc.sync.dma_start(
                    out=y_h[t0 + CH * chh:t0 + CH * (chh + 1),
                            (C // 2) * half:(C // 2) * (half + 1)],
                    in_=yst)
    ctx.close()


def kernel(**inputs):
    from concourse.bass_utils import run_bass_kernel_spmd
    if "nc" not in _CACHE:
        _CACHE["nc"] = _build()
    nc = _CACHE["nc"]
    x = np.ascontiguousarray(np.asarray(inputs["x"], dtype=np.float32))
    shared = {n: np.ascontiguousarray(np.asarray(inputs[n], dtype=np.float32))
              for n in MAT_NAMES + VEC_NAMES}
    in_maps = [dict(shared, x=np.ascontiguousarray(x[b])) for b in range(B)]
    res = run_bass_kernel_spmd(nc, in_maps, core_ids=list(range(B)))
    y = np.stack([res.results[b]["y"] for b in range(B)])
    vp = np.stack([res.results[b]["vp"] for b in range(B)])
    return y, vp
